# revision 13
# baseline (speedup 1.0000x reference)
"""Trainium2 Bass kernel for nn_LogicLayer.

Math: out[b,o] = sum_f softmax(weights[o])[f] * op_f(a,b),
      a = x[b, idx0[o]], b = x[b, idx1[o]].
All 16 logic ops are affine in {1, a, b, ab}, so
      out[b,o] = C0[o] + CA[o]*a + CB[o]*b + CAB[o]*a*b
with per-neuron coefficients Cj[o] = sum_f probs[o,f] * T[f,j].

Strategy (8 NeuronCores, out_dim sharded 8192 neurons/core):
 - Host: transpose x -> x_T [IN_DIM, B] in fp16 so a gathered "column of x"
   is a contiguous 512B row; split into two 32768-row halves (dma_gather
   uses int16 indices, max 32768 rows).
 - Per core, bucket its 8192 columns by (half(idx0), half(idx1)) so each
   dma_gather call reads one half with int16 indices; pad buckets to a
   multiple of 128 with index 0 (valid row; padded outputs dropped on host).
 - Device: SWDGE dma_gather rows of x_T into SBUF [128, slots, 256] fp16,
   rotating calls across all 4 SWDGE queues (each queue runs on its own Q7
   core pair and has its own descriptor ring, so emission parallelizes and
   ring-space waits don't serialize calls).
 - Softmax+coefficient reduction in f32 on Scalar/Vector engines, then per
   slot: ACT computes t1 = CAB*a + CB, DVE computes t2 = CA*a + C0
   (tensor_scalar with two per-partition scalars), and chunk-wide DVE does
   out = t1*b + t2. Output written back as fp16, upcast on host.
 - Host: invert the bucket permutation and transpose back to [B, OUT_DIM].
"""

import os

import numpy as np

B = 256
IN_DIM = 65536
OUT_DIM = 65536
NFN = 16
NCORES = 8
SHARD = OUT_DIM // NCORES
HALF = IN_DIM // 2
P = 128

# Coefficient table: op_f(a,b) = T[f,0] + T[f,1]*a + T[f,2]*b + T[f,3]*ab
_T = np.array(
    [
        [0, 0, 0, 0],    # false
        [0, 0, 0, 1],    # a AND b
        [0, 1, 0, -1],   # a AND NOT b
        [0, 1, 0, 0],    # a
        [0, 0, 1, -1],   # NOT a AND b
        [0, 0, 1, 0],    # b
        [0, 1, 1, -2],   # XOR
        [0, 1, 1, -1],   # OR
        [1, -1, -1, 1],  # NOR
        [1, -1, -1, 2],  # XNOR
        [1, 0, -1, 0],   # NOT b
        [1, 0, -1, 1],   # a OR NOT b
        [1, -1, 0, 0],   # NOT a
        [1, -1, 0, 1],   # NOT a OR b
        [1, 0, 0, -1],   # NAND
        [1, 0, 0, 0],    # true
    ],
    dtype=np.float32,
)

_BUILD_CACHE = {}
LAST_RESULTS = None  # BassKernelResults of the most recent run (for profiling)


def _wrap_idx(idx16):
    """[n] int16 -> [128, n//16] wrapped: position i at (i%16, i//16),
    replicated across the 8 groups of 16 partitions (one per Q7 core)."""
    w = idx16.reshape(-1, 16).T  # [16, n/16]
    return np.ascontiguousarray(np.tile(w, (8, 1)))


def _build_kernel(caps):
    """Build + compile the SPMD program for bucket capacities `caps` (4-tuple,
    each a multiple of 128). Returns (nc, npad)."""
    key = tuple(caps)
    if key in _BUILD_CACHE:
        return _BUILD_CACHE[key]

    import concourse.bacc as bacc
    import concourse.mybir as mybir
    import concourse.tile as tile
    from concourse import library_config

    npad = int(sum(caps))
    nslot = npad // P
    offs = np.concatenate([[0], np.cumsum(caps)]).astype(int)

    nc = bacc.Bacc(
        "TRN2",
        target_bir_lowering=False,
        debug=False,
        dynamic_dma_scratch_size=int(os.environ.get("K_DMA_SCRATCH", "16384")),
        num_swdge_queues=4,
    )
    f32 = mybir.dt.float32
    f16 = mybir.dt.float16
    i16 = mybir.dt.int16

    xA_d = nc.dram_tensor("xA", [HALF, B], f16, kind="ExternalInput")
    xB_d = nc.dram_tensor("xB", [HALF, B], f16, kind="ExternalInput")
    ia_d = nc.dram_tensor("ia", [P, npad // 16], i16, kind="ExternalInput")
    ib_d = nc.dram_tensor("ib", [P, npad // 16], i16, kind="ExternalInput")
    # w pre-wrapped on host to [P, nslot, NFN] so the load is contiguous
    w_d = nc.dram_tensor("w", [P, nslot * NFN], f32, kind="ExternalInput")
    out_d = nc.dram_tensor("out", [P, nslot * B], f16, kind="ExternalOutput")

    Exp = mybir.ActivationFunctionType.Exp
    Ident = mybir.ActivationFunctionType.Identity
    X = mybir.AxisListType.X
    Mult = mybir.AluOpType.mult
    Add = mybir.AluOpType.add

    # per-chunk gather call ranges: split [c0, c1) at bucket boundaries and
    # cap each call at MAX_CALL indices.
    MAX_CALL = int(os.environ.get("K_MAX_CALL", "1024"))

    def call_ranges(c0, c1):
        out = []
        for k in range(4):
            lo, hi = max(c0, offs[k]), min(c1, offs[k + 1])
            while lo < hi:
                m = min(hi, lo + MAX_CALL)
                out.append((lo, m, k))
                lo = m
        return out

    from contextlib import ExitStack

    with tile.TileContext(nc) as tc, ExitStack() as ctx:
        nc.gpsimd.load_library(library_config.mlp)
        consts = ctx.enter_context(tc.tile_pool(name="consts", bufs=1))
        work = ctx.enter_context(
            tc.tile_pool(name="work", bufs=int(os.environ.get("K_BUFS", "4")))
        )

        # --- load index lists (stay resident) ---
        ia_t = consts.tile([P, npad // 16], i16)
        ib_t = consts.tile([P, npad // 16], i16)
        nc.sync.dma_start(out=ia_t[:], in_=ia_d[:])
        nc.sync.dma_start(out=ib_t[:], in_=ib_d[:])

        # --- warmup: one tiny gather absorbs the Q7 library IRAM load while
        # the index DMAs land ---
        warm_i = consts.tile([P, 8], i16)
        nc.gpsimd.memset(warm_i[:], 0)
        warm_o = consts.tile([P, 1, B], f16)
        nc.gpsimd.dma_gather(
            out_ap=warm_o[:],
            in_ap=xA_d[:],
            idxs_ap=warm_i[:],
            num_idxs=P,
            num_idxs_reg=P,
            elem_size=B,
            single_packet=True,
            queue_num=0,
        )

        # --- softmax -> affine coefficients for all positions ---
        w_t = consts.tile([P, nslot * NFN], f32)
        nc.sync.dma_start(out=w_t[:], in_=w_d[:])
        e_t = consts.tile([P, nslot * NFN], f32)
        nc.scalar.activation(e_t[:], w_t[:], Exp)
        e3 = e_t[:].rearrange("p (s f) -> p s f", f=NFN)

        def rsum(dst, src_ap):
            nc.vector.tensor_reduce(dst, src_ap, axis=X, op=mybir.AluOpType.add)

        s_t = consts.tile([P, nslot], f32)     # sum_f e
        rden = consts.tile([P, nslot], f32)    # 1/sum
        c0_t = consts.tile([P, nslot], f32)
        ca_t = consts.tile([P, nslot], f32)
        cb_t = consts.tile([P, nslot], f32)
        cab_t = consts.tile([P, nslot], f32)
        tmp1 = consts.tile([P, nslot], f32)
        tmp2 = consts.tile([P, nslot], f32)

        rsum(s_t[:], e3)
        nc.vector.reciprocal(out=rden[:], in_=s_t[:])

        # C0: +{8..15}
        rsum(c0_t[:], e3[:, :, 8:16])
        # CA: +{2,3} +{6,7} -{8,9} -{12,13}
        rsum(ca_t[:], e3[:, :, 2:4])
        rsum(tmp1[:], e3[:, :, 6:8])
        nc.vector.tensor_add(ca_t[:], ca_t[:], tmp1[:])
        rsum(tmp1[:], e3[:, :, 8:10])
        nc.vector.tensor_sub(ca_t[:], ca_t[:], tmp1[:])
        rsum(tmp1[:], e3[:, :, 12:14])
        nc.vector.tensor_sub(ca_t[:], ca_t[:], tmp1[:])
        # CB: +{4..7} -{8..11}
        rsum(cb_t[:], e3[:, :, 4:8])
        rsum(tmp1[:], e3[:, :, 8:12])
        nc.vector.tensor_sub(cb_t[:], cb_t[:], tmp1[:])
        # CAB: +e1 -e2 -e4 -2*e6 -e7 +e8 +2*e9 +e11 +e13 -e14
        #    = (e1+e8+e11+e13) - (e2+e4+e7+e14) + 2*(e9-e6)
        def ef(f):
            return e3[:, :, f]

        nc.vector.tensor_add(cab_t[:], ef(1), ef(8))
        nc.vector.tensor_add(cab_t[:], cab_t[:], ef(11))
        nc.vector.tensor_add(cab_t[:], cab_t[:], ef(13))
        nc.vector.tensor_add(tmp1[:], ef(2), ef(4))
        nc.vector.tensor_add(tmp1[:], tmp1[:], ef(7))
        nc.vector.tensor_add(tmp1[:], tmp1[:], ef(14))
        nc.vector.tensor_sub(cab_t[:], cab_t[:], tmp1[:])
        nc.vector.tensor_sub(tmp2[:], ef(9), ef(6))
        nc.vector.tensor_add(cab_t[:], cab_t[:], tmp2[:])
        nc.vector.tensor_add(cab_t[:], cab_t[:], tmp2[:])
        # normalize
        for ct in (c0_t, ca_t, cb_t, cab_t):
            nc.vector.tensor_mul(ct[:], ct[:], rden[:])
        # f16 copies of CA/C0 for chunk-wide broadcast ops on DVE
        cah_t = consts.tile([P, nslot], f16)
        c0h_t = consts.tile([P, nslot], f16)
        nc.vector.tensor_copy(cah_t[:], ca_t[:])
        nc.vector.tensor_copy(c0h_t[:], c0_t[:])

        # --- main loop over bucket-aligned chunks of columns ---
        # each bucket is split into equal-sized chunks (multiples of 128) so
        # every gather call is the same size: no tiny remainder calls that pay
        # full completion-latency periods.
        chunk_cap = int(os.environ.get("K_CHUNK_POS", "1024"))
        chunks = []
        for k in range(4):
            cap = int(caps[k])
            if cap == 0:
                continue
            nch = max(1, -(-cap // chunk_cap))
            lo = int(offs[k])
            for i in range(nch):
                sz = P * (
                    (cap * (i + 1)) // (nch * P) - (cap * i) // (nch * P)
                )
                chunks.append((lo, lo + sz))
                lo += sz
            assert lo == offs[k + 1]
        qrot = 0
        for (p0g, p1g) in chunks:
            cbase, cs = p0g // P, (p1g - p0g) // P
            a_t = work.tile([P, cs, B], f16)
            b_t = work.tile([P, cs, B], f16)
            t1_t = work.tile([P, cs, B], f16)
            t2_t = work.tile([P, cs, B], f16)
            o_t = work.tile([P, cs * B], f16)
            for (lo, hi, k) in call_ranges(p0g, p1g):
                n = hi - lo
                src = xA_d if k < 2 else xB_d
                srcb = xA_d if k % 2 == 0 else xB_d
                sl = (lo - p0g) // P
                sh = (hi - p0g) // P
                nc.gpsimd.dma_gather(
                    out_ap=a_t[:, sl:sh, :],
                    in_ap=src[:],
                    idxs_ap=ia_t[:, lo // 16 : hi // 16],
                    num_idxs=n,
                    num_idxs_reg=n,
                    elem_size=B,
                    single_packet=True,
                    queue_num=qrot % 4,
                )
                qrot += 1
                nc.gpsimd.dma_gather(
                    out_ap=b_t[:, sl:sh, :],
                    in_ap=srcb[:],
                    idxs_ap=ib_t[:, lo // 16 : hi // 16],
                    num_idxs=n,
                    num_idxs_reg=n,
                    elem_size=B,
                    single_packet=True,
                    queue_num=qrot % 4,
                )
                qrot += 1
            o3 = o_t[:].rearrange("p (s e) -> p s e", e=B)
            for s in range(cs):
                g = cbase + s
                # t1 = CAB*a + CB   (ScalarE, per-partition scale/bias)
                nc.scalar.activation(
                    t1_t[:, s, :], a_t[:, s, :], Ident,
                    bias=cb_t[:, g : g + 1], scale=cab_t[:, g : g + 1],
                )
            # out = t1*b + CA*a + C0  (VectorE chunk-wide; CA/C0 broadcast
            # along batch via stride-0 APs)
            ca_b = (
                cah_t[:, cbase : cbase + cs].unsqueeze(2).broadcast_to([P, cs, B])
            )
            c0_b = (
                c0h_t[:, cbase : cbase + cs].unsqueeze(2).broadcast_to([P, cs, B])
            )
            nc.vector.tensor_mul(o3[:], t1_t[:], b_t[:])
            nc.vector.tensor_mul(t2_t[:], a_t[:], ca_b)
            nc.vector.tensor_add(o3[:], o3[:], t2_t[:])
            nc.vector.tensor_add(o3[:], o3[:], c0_b)
            nc.sync.dma_start(
                out=out_d[:, cbase * B : (cbase + cs) * B], in_=o_t[:]
            )

    nc.compile()
    _BUILD_CACHE[key] = (nc, npad)
    return nc, npad


def kernel(x, weights, indices):
    from concourse.bass_utils import run_bass_kernel_spmd

    x = np.asarray(x, dtype=np.float32)
    weights = np.asarray(weights, dtype=np.float32)
    indices = np.asarray(indices, dtype=np.int64)

    x_T = np.ascontiguousarray(x.T.astype(np.float16))  # [IN_DIM, B] fp16
    xA = x_T[:HALF]
    xB = x_T[HALF:]

    # --- per-core bucketing ---
    percore = []
    counts_all = np.zeros((NCORES, 4), dtype=np.int64)
    for c in range(NCORES):
        sl = slice(c * SHARD, (c + 1) * SHARD)
        i0 = indices[0, sl]
        i1 = indices[1, sl]
        bid = (i0 >= HALF).astype(np.int64) * 2 + (i1 >= HALF).astype(np.int64)
        order = np.argsort(bid, kind="stable")
        counts = np.bincount(bid, minlength=4)
        counts_all[c] = counts
        percore.append((sl, i0, i1, bid, order, counts))

    caps = tuple(
        int(-(-int(counts_all[:, k].max()) // P) * P) for k in range(4)
    )
    nc, npad = _build_kernel(caps)
    nslot = npad // P
    offs = np.concatenate([[0], np.cumsum(caps)]).astype(int)

    in_maps = []
    pos_maps = []  # per core: global column index per position (-1 = pad)
    for c in range(NCORES):
        sl, i0, i1, bid, order, counts = percore[c]
        ia = np.zeros(npad, dtype=np.int16)
        ib = np.zeros(npad, dtype=np.int16)
        pos = np.full(npad, -1, dtype=np.int64)
        w_pad = np.zeros((npad, NFN), dtype=np.float32)
        w_shard = weights[sl]
        for k in range(4):
            selk = order[np.searchsorted(bid[order], k) :][: counts[k]]
            o, n = int(offs[k]), int(counts[k])
            ia[o : o + n] = (i0[selk] - (HALF if k >= 2 else 0)).astype(np.int16)
            ib[o : o + n] = (i1[selk] - (HALF if k % 2 else 0)).astype(np.int16)
            pos[o : o + n] = sl.start + selk
            w_pad[o : o + n] = w_shard[selk]
        # wrap to [P, nslot*NFN]: position i = s*128 + p -> partition p, slot s
        w_wrapped = np.ascontiguousarray(
            w_pad.reshape(nslot, P, NFN).transpose(1, 0, 2)
        ).reshape(P, nslot * NFN)
        in_maps.append(
            {
                "xA": xA,
                "xB": xB,
                "ia": _wrap_idx(ia),
                "ib": _wrap_idx(ib),
                "w": w_wrapped,
            }
        )
        pos_maps.append(pos)

    res = run_bass_kernel_spmd(nc, in_maps, core_ids=list(range(NCORES)))
    global LAST_RESULTS
    LAST_RESULTS = res

    out = np.empty((B, OUT_DIM), dtype=np.float32)
    for c in range(NCORES):
        o = res.results[c]["out"].reshape(P, nslot, B).astype(np.float32)
        rows = np.ascontiguousarray(o.transpose(1, 0, 2)).reshape(npad, B)
        pos = pos_maps[c]
        valid = pos >= 0
        out[:, pos[valid]] = rows[valid].T
    return out


# revision 14
# speedup vs baseline: 1.1655x; 1.1655x over previous
"""Trainium2 Bass kernel for nn_LogicLayer.

Math: out[b,o] = sum_f softmax(weights[o])[f] * op_f(a,b),
      a = x[b, idx0[o]], b = x[b, idx1[o]].
All 16 logic ops are affine in {1, a, b, ab}, so
      out[b,o] = C0[o] + CA[o]*a + CB[o]*b + CAB[o]*a*b
with per-neuron coefficients Cj[o] = sum_f probs[o,f] * T[f,j].

Strategy (8 NeuronCores, out_dim sharded 8192 neurons/core):
 - Host: transpose x -> x_T [IN_DIM, B] in fp16 so a gathered "column of x"
   is a contiguous 512B row; split into two 32768-row halves (dma_gather
   uses int16 indices, max 32768 rows).
 - Per core, bucket its 8192 columns by (half(idx0), half(idx1)) so each
   dma_gather call reads one half with int16 indices; pad buckets to a
   multiple of 128 with index 0 (valid row; padded outputs dropped on host).
 - Device: SWDGE dma_gather rows of x_T into SBUF [128, slots, 256] fp16,
   rotating calls across all 4 SWDGE queues (each queue runs on its own Q7
   core pair and has its own descriptor ring, so emission parallelizes and
   ring-space waits don't serialize calls).
 - Softmax+coefficient reduction in f32 on Scalar/Vector engines, then per
   slot: ACT computes t1 = CAB*a + CB, DVE computes t2 = CA*a + C0
   (tensor_scalar with two per-partition scalars), and chunk-wide DVE does
   out = t1*b + t2. Output written back as fp16, upcast on host.
 - Host: invert the bucket permutation and transpose back to [B, OUT_DIM].
"""

import os

import numpy as np

B = 256
IN_DIM = 65536
OUT_DIM = 65536
NFN = 16
NCORES = 8
SHARD = OUT_DIM // NCORES
HALF = IN_DIM // 2
P = 128

# Coefficient table: op_f(a,b) = T[f,0] + T[f,1]*a + T[f,2]*b + T[f,3]*ab
_T = np.array(
    [
        [0, 0, 0, 0],    # false
        [0, 0, 0, 1],    # a AND b
        [0, 1, 0, -1],   # a AND NOT b
        [0, 1, 0, 0],    # a
        [0, 0, 1, -1],   # NOT a AND b
        [0, 0, 1, 0],    # b
        [0, 1, 1, -2],   # XOR
        [0, 1, 1, -1],   # OR
        [1, -1, -1, 1],  # NOR
        [1, -1, -1, 2],  # XNOR
        [1, 0, -1, 0],   # NOT b
        [1, 0, -1, 1],   # a OR NOT b
        [1, -1, 0, 0],   # NOT a
        [1, -1, 0, 1],   # NOT a OR b
        [1, 0, 0, -1],   # NAND
        [1, 0, 0, 0],    # true
    ],
    dtype=np.float32,
)

_BUILD_CACHE = {}
LAST_RESULTS = None  # BassKernelResults of the most recent run (for profiling)


def _wrap_idx(idx16):
    """[n] int16 -> [128, n//16] wrapped: position i at (i%16, i//16),
    replicated across the 8 groups of 16 partitions (one per Q7 core)."""
    w = idx16.reshape(-1, 16).T  # [16, n/16]
    return np.ascontiguousarray(np.tile(w, (8, 1)))


def _build_kernel(caps):
    """Build + compile the SPMD program for bucket capacities `caps` (4-tuple,
    each a multiple of 128). Returns (nc, npad)."""
    key = tuple(caps)
    if key in _BUILD_CACHE:
        return _BUILD_CACHE[key]

    import concourse.bacc as bacc
    import concourse.mybir as mybir
    import concourse.tile as tile
    from concourse import library_config

    npad = int(sum(caps))
    nslot = npad // P
    offs = np.concatenate([[0], np.cumsum(caps)]).astype(int)

    nc = bacc.Bacc(
        "TRN2",
        target_bir_lowering=False,
        debug=False,
        dynamic_dma_scratch_size=int(os.environ.get("K_DMA_SCRATCH", "16384")),
        num_swdge_queues=4,
    )
    f32 = mybir.dt.float32
    f16 = mybir.dt.float16
    i16 = mybir.dt.int16

    xA_d = nc.dram_tensor("xA", [HALF, B], f16, kind="ExternalInput")
    xB_d = nc.dram_tensor("xB", [HALF, B], f16, kind="ExternalInput")
    ia_d = nc.dram_tensor("ia", [P, npad // 16], i16, kind="ExternalInput")
    ib_d = nc.dram_tensor("ib", [P, npad // 16], i16, kind="ExternalInput")
    # w pre-wrapped on host to [P, nslot, NFN] so the load is contiguous
    w_d = nc.dram_tensor("w", [P, nslot * NFN], f32, kind="ExternalInput")
    out_d = nc.dram_tensor("out", [P, nslot * B], f16, kind="ExternalOutput")

    Exp = mybir.ActivationFunctionType.Exp
    Ident = mybir.ActivationFunctionType.Identity
    X = mybir.AxisListType.X
    Mult = mybir.AluOpType.mult
    Add = mybir.AluOpType.add

    # per-chunk gather call ranges: split [c0, c1) at bucket boundaries and
    # cap each call at MAX_CALL indices.
    MAX_CALL = int(os.environ.get("K_MAX_CALL", "1024"))

    def call_ranges(c0, c1):
        out = []
        for k in range(4):
            lo, hi = max(c0, offs[k]), min(c1, offs[k + 1])
            while lo < hi:
                m = min(hi, lo + MAX_CALL)
                out.append((lo, m, k))
                lo = m
        return out

    from contextlib import ExitStack

    with tile.TileContext(nc) as tc, ExitStack() as ctx:
        nc.gpsimd.load_library(library_config.mlp)
        consts = ctx.enter_context(tc.tile_pool(name="consts", bufs=1))
        work = ctx.enter_context(
            tc.tile_pool(name="work", bufs=int(os.environ.get("K_BUFS", "4")))
        )

        # --- load index lists (stay resident) ---
        ia_t = consts.tile([P, npad // 16], i16)
        ib_t = consts.tile([P, npad // 16], i16)
        nc.sync.dma_start(out=ia_t[:], in_=ia_d[:])
        nc.sync.dma_start(out=ib_t[:], in_=ib_d[:])

        # --- warmup: one tiny gather absorbs the Q7 library IRAM load while
        # the index DMAs land ---
        warm_i = consts.tile([P, 8], i16)
        nc.gpsimd.memset(warm_i[:], 0)
        warm_o = consts.tile([P, 1, B], f16)
        nc.gpsimd.dma_gather(
            out_ap=warm_o[:],
            in_ap=xA_d[:],
            idxs_ap=warm_i[:],
            num_idxs=P,
            num_idxs_reg=P,
            elem_size=B,
            single_packet=True,
            queue_num=0,
        )

        # --- softmax -> affine coefficients for all positions ---
        w_t = consts.tile([P, nslot * NFN], f32)
        nc.sync.dma_start(out=w_t[:], in_=w_d[:])
        e_t = consts.tile([P, nslot * NFN], f32)
        nc.scalar.activation(e_t[:], w_t[:], Exp)
        e3 = e_t[:].rearrange("p (s f) -> p s f", f=NFN)

        def rsum(dst, src_ap):
            nc.vector.tensor_reduce(dst, src_ap, axis=X, op=mybir.AluOpType.add)

        s_t = consts.tile([P, nslot], f32)     # sum_f e
        rden = consts.tile([P, nslot], f32)    # 1/sum
        c0_t = consts.tile([P, nslot], f32)
        ca_t = consts.tile([P, nslot], f32)
        cb_t = consts.tile([P, nslot], f32)
        cab_t = consts.tile([P, nslot], f32)
        tmp1 = consts.tile([P, nslot], f32)
        tmp2 = consts.tile([P, nslot], f32)

        rsum(s_t[:], e3)
        nc.vector.reciprocal(out=rden[:], in_=s_t[:])

        # C0: +{8..15}
        rsum(c0_t[:], e3[:, :, 8:16])
        # CA: +{2,3} +{6,7} -{8,9} -{12,13}
        rsum(ca_t[:], e3[:, :, 2:4])
        rsum(tmp1[:], e3[:, :, 6:8])
        nc.vector.tensor_add(ca_t[:], ca_t[:], tmp1[:])
        rsum(tmp1[:], e3[:, :, 8:10])
        nc.vector.tensor_sub(ca_t[:], ca_t[:], tmp1[:])
        rsum(tmp1[:], e3[:, :, 12:14])
        nc.vector.tensor_sub(ca_t[:], ca_t[:], tmp1[:])
        # CB: +{4..7} -{8..11}
        rsum(cb_t[:], e3[:, :, 4:8])
        rsum(tmp1[:], e3[:, :, 8:12])
        nc.vector.tensor_sub(cb_t[:], cb_t[:], tmp1[:])
        # CAB: +e1 -e2 -e4 -2*e6 -e7 +e8 +2*e9 +e11 +e13 -e14
        #    = (e1+e8+e11+e13) - (e2+e4+e7+e14) + 2*(e9-e6)
        def ef(f):
            return e3[:, :, f]

        nc.vector.tensor_add(cab_t[:], ef(1), ef(8))
        nc.vector.tensor_add(cab_t[:], cab_t[:], ef(11))
        nc.vector.tensor_add(cab_t[:], cab_t[:], ef(13))
        nc.vector.tensor_add(tmp1[:], ef(2), ef(4))
        nc.vector.tensor_add(tmp1[:], tmp1[:], ef(7))
        nc.vector.tensor_add(tmp1[:], tmp1[:], ef(14))
        nc.vector.tensor_sub(cab_t[:], cab_t[:], tmp1[:])
        nc.vector.tensor_sub(tmp2[:], ef(9), ef(6))
        nc.vector.tensor_add(cab_t[:], cab_t[:], tmp2[:])
        nc.vector.tensor_add(cab_t[:], cab_t[:], tmp2[:])
        # normalize
        for ct in (c0_t, ca_t, cb_t, cab_t):
            nc.vector.tensor_mul(ct[:], ct[:], rden[:])
        # f16 copies of CA/C0 for chunk-wide broadcast ops on DVE
        cah_t = consts.tile([P, nslot], f16)
        c0h_t = consts.tile([P, nslot], f16)
        nc.vector.tensor_copy(cah_t[:], ca_t[:])
        nc.vector.tensor_copy(c0h_t[:], c0_t[:])

        # --- main loop over bucket-aligned chunks of columns ---
        # each bucket is split into equal-sized chunks (multiples of 128) so
        # every gather call is the same size: no tiny remainder calls that pay
        # full completion-latency periods.
        chunk_cap = int(os.environ.get("K_CHUNK_POS", "1024"))
        chunks = []
        for k in range(4):
            cap = int(caps[k])
            if cap == 0:
                continue
            nch = max(1, -(-cap // chunk_cap))
            lo = int(offs[k])
            for i in range(nch):
                sz = P * (
                    (cap * (i + 1)) // (nch * P) - (cap * i) // (nch * P)
                )
                chunks.append((lo, lo + sz))
                lo += sz
            assert lo == offs[k + 1]
        qrot = 0
        for (p0g, p1g) in chunks:
            cbase, cs = p0g // P, (p1g - p0g) // P
            a_t = work.tile([P, cs, B], f16)
            b_t = work.tile([P, cs, B], f16)
            t1_t = work.tile([P, cs, B], f16)
            t2_t = work.tile([P, cs, B], f16)
            o_t = work.tile([P, cs * B], f16)
            for (lo, hi, k) in call_ranges(p0g, p1g):
                n = hi - lo
                src = xA_d if k < 2 else xB_d
                srcb = xA_d if k % 2 == 0 else xB_d
                sl = (lo - p0g) // P
                sh = (hi - p0g) // P
                nc.gpsimd.dma_gather(
                    out_ap=a_t[:, sl:sh, :],
                    in_ap=src[:],
                    idxs_ap=ia_t[:, lo // 16 : hi // 16],
                    num_idxs=n,
                    num_idxs_reg=n,
                    elem_size=B,
                    single_packet=True,
                    queue_num=qrot % 4,
                )
                qrot += 1
                nc.gpsimd.dma_gather(
                    out_ap=b_t[:, sl:sh, :],
                    in_ap=srcb[:],
                    idxs_ap=ib_t[:, lo // 16 : hi // 16],
                    num_idxs=n,
                    num_idxs_reg=n,
                    elem_size=B,
                    single_packet=True,
                    queue_num=qrot % 4,
                )
                qrot += 1
            o3 = o_t[:].rearrange("p (s e) -> p s e", e=B)
            for s in range(cs):
                g = cbase + s
                # t1 = CAB*a + CB   (ScalarE, per-partition scale/bias)
                nc.scalar.activation(
                    t1_t[:, s, :], a_t[:, s, :], Ident,
                    bias=cb_t[:, g : g + 1], scale=cab_t[:, g : g + 1],
                )
                # t2 = CA*a + C0, alternating between ScalarE and VectorE to
                # balance the two engines' per-slot instruction overheads
                if s % 2 == 0:
                    nc.vector.tensor_scalar(
                        t2_t[:, s, :], a_t[:, s, :],
                        ca_t[:, g : g + 1], c0_t[:, g : g + 1],
                        Mult, Add,
                    )
                else:
                    nc.scalar.activation(
                        t2_t[:, s, :], a_t[:, s, :], Ident,
                        bias=c0_t[:, g : g + 1], scale=ca_t[:, g : g + 1],
                    )
            # out = t1*b + t2   (VectorE, chunk-wide)
            nc.vector.tensor_mul(o3[:], t1_t[:], b_t[:])
            nc.vector.tensor_add(o3[:], o3[:], t2_t[:])
            nc.sync.dma_start(
                out=out_d[:, cbase * B : (cbase + cs) * B], in_=o_t[:]
            )

    nc.compile()
    _BUILD_CACHE[key] = (nc, npad)
    return nc, npad


def kernel(x, weights, indices):
    from concourse.bass_utils import run_bass_kernel_spmd

    x = np.asarray(x, dtype=np.float32)
    weights = np.asarray(weights, dtype=np.float32)
    indices = np.asarray(indices, dtype=np.int64)

    x_T = np.ascontiguousarray(x.T.astype(np.float16))  # [IN_DIM, B] fp16
    xA = x_T[:HALF]
    xB = x_T[HALF:]

    # --- per-core bucketing ---
    percore = []
    counts_all = np.zeros((NCORES, 4), dtype=np.int64)
    for c in range(NCORES):
        sl = slice(c * SHARD, (c + 1) * SHARD)
        i0 = indices[0, sl]
        i1 = indices[1, sl]
        bid = (i0 >= HALF).astype(np.int64) * 2 + (i1 >= HALF).astype(np.int64)
        order = np.argsort(bid, kind="stable")
        counts = np.bincount(bid, minlength=4)
        counts_all[c] = counts
        percore.append((sl, i0, i1, bid, order, counts))

    caps = tuple(
        int(-(-int(counts_all[:, k].max()) // P) * P) for k in range(4)
    )
    nc, npad = _build_kernel(caps)
    nslot = npad // P
    offs = np.concatenate([[0], np.cumsum(caps)]).astype(int)

    in_maps = []
    pos_maps = []  # per core: global column index per position (-1 = pad)
    for c in range(NCORES):
        sl, i0, i1, bid, order, counts = percore[c]
        ia = np.zeros(npad, dtype=np.int16)
        ib = np.zeros(npad, dtype=np.int16)
        pos = np.full(npad, -1, dtype=np.int64)
        w_pad = np.zeros((npad, NFN), dtype=np.float32)
        w_shard = weights[sl]
        for k in range(4):
            selk = order[np.searchsorted(bid[order], k) :][: counts[k]]
            o, n = int(offs[k]), int(counts[k])
            ia[o : o + n] = (i0[selk] - (HALF if k >= 2 else 0)).astype(np.int16)
            ib[o : o + n] = (i1[selk] - (HALF if k % 2 else 0)).astype(np.int16)
            pos[o : o + n] = sl.start + selk
            w_pad[o : o + n] = w_shard[selk]
        # wrap to [P, nslot*NFN]: position i = s*128 + p -> partition p, slot s
        w_wrapped = np.ascontiguousarray(
            w_pad.reshape(nslot, P, NFN).transpose(1, 0, 2)
        ).reshape(P, nslot * NFN)
        in_maps.append(
            {
                "xA": xA,
                "xB": xB,
                "ia": _wrap_idx(ia),
                "ib": _wrap_idx(ib),
                "w": w_wrapped,
            }
        )
        pos_maps.append(pos)

    res = run_bass_kernel_spmd(nc, in_maps, core_ids=list(range(NCORES)))
    global LAST_RESULTS
    LAST_RESULTS = res

    out = np.empty((B, OUT_DIM), dtype=np.float32)
    for c in range(NCORES):
        o = res.results[c]["out"].reshape(P, nslot, B).astype(np.float32)
        rows = np.ascontiguousarray(o.transpose(1, 0, 2)).reshape(npad, B)
        pos = pos_maps[c]
        valid = pos >= 0
        out[:, pos[valid]] = rows[valid].T
    return out
